# revision 1
# baseline (speedup 1.0000x reference)
"""GCNConv (COO SpMM + feature transform) distributed over 8 NeuronCores.

out = segment_sum(x[cols] * vals, rows) @ weight

Strategy (1D row partition of the sparse matrix, per the CAGNET-style hint):
 - Destination rows are split into 8 contiguous blocks of 12500 rows; core k
   owns rows [12500k, 12500(k+1)) and the edges targeting them (edges arrive
   sorted by destination row).
 - x (the gather table) and the 32x32 weight are replicated per core.
 - Host-side (inside kernel(), numpy): each core's rows are bin-packed into
   "tiles" of <=128 edge slots / <=M_FIX rows.  For each tile we build
     idx[p]  : source node of edge-slot p   (gather index)
     bval[p, i] = val(edge) if slot p belongs to tile-row i else 0
   i.e. bval is the one-hot segment-sum matrix with the edge weights folded
   in, fully precomputed on host.
 - Device: per tile, one indirect DMA (the only HW-supported gather mode on
   this image: 128 per-partition offsets, one 128B x-row per partition)
   pulls the tile's 128 source rows, then one matmul
       zT[32, t*M:(t+1)*M] = gath[128,32].T @ bval[128,M]
   does the val-weighted segment-sum on the TensorEngine.  Per super-block
   of TPS tiles the finished zT[32, 512] is copied out of PSUM and hit with
   the weight (out = zT.T @ W per 128-row chunk — no transposes needed),
   then one DMA stores the 512 finished rows.  The kernel is bound by the
   GpSimd SWDGE descriptor-generation rate (~1.4us per 128-row gather).
 - Host un-permutes the packed fragments into the final [100000, 32] output
   (rows split across fragments are summed).
"""

import os
import sys
import tempfile
import types

import numpy as np

# A transiently-wedged device can leave a poisoned NEFF in the shared neuron
# compile cache, making every later invocation with the same cache key crash
# (observed: NRT_EXEC_UNIT_UNRECOVERABLE on known-good programs).  Compiling
# is only a few seconds here, so use a fresh per-process cache instead.
os.environ["NEURON_COMPILE_CACHE_URL"] = tempfile.mkdtemp(prefix="neuron-cc-cache-")


def _install_ntff_hook_shim():
    """bass_utils' axon trace path imports antenv.axon_hooks, which this
    container image lacks.  Provide it (with the real ctypes-based profiler
    hook when available) so BASS_TRACE=1 in the environment doesn't crash."""
    if "antenv.axon_hooks" in sys.modules:
        return
    mod = types.ModuleType("antenv.axon_hooks")
    _h = [None]
    mod.set_axon_ntff_profile_hook = lambda h: _h.__setitem__(0, h)
    mod.get_axon_ntff_profile_hook = lambda: _h[0]
    sys.modules["antenv.axon_hooks"] = mod
    try:
        from trn_agent_boot.trn_boot import _ntff_profile_via_ctypes

        mod.set_axon_ntff_profile_hook(
            _ntff_profile_via_ctypes("/opt/axon/libaxon_pjrt.so")
        )
    except Exception:
        pass


_install_ntff_hook_shim()

import concourse.bass as bass
import concourse.mybir as mybir
import concourse.tile as tile
from concourse import bacc
from concourse.bass import IndirectOffsetOnAxis
from concourse.bass_utils import run_bass_kernel_spmd

N_NODES = 100_000
N_CORES = 8
RPC = N_NODES // N_CORES  # rows per core
F = 32
M_FIX = 16                # output rows (bval columns) per tile
TPS = 8                   # tiles per super-block
RPS = M_FIX * TPS         # 512 output rows per super-block
P = 128

f32 = mybir.dt.float32
i32 = mybir.dt.int32

_compiled_cache = {}


DMAX = 120   # max slots per item (bigger rows split into fragments)
CROWS = 10   # max rows merged into one shared-col cluster
CSLOT = 118  # max pre-dedup slot budget of a cluster


class _Item:
    """A packable unit: `slots` (source cols to gather, one per slot) and
    `rows` = [(local_row, slot_idx_array, val_array)].  Clusters dedupe cols
    shared between their rows (one gather slot feeds several bval columns);
    single-row items are splittable for tile top-off."""

    __slots__ = ("slots", "rows", "splittable")

    def __init__(self, slots, rows, splittable):
        self.slots = slots
        self.rows = rows
        self.splittable = splittable


def _cluster_rows(d, starts, cols, vals):
    """Union rows sharing source cols (caps: CROWS rows, CSLOT total edges)."""
    nrow = len(d)
    parent = np.arange(nrow)
    csize = d.copy()          # total edges in cluster
    crows = np.ones(nrow, np.int64)

    def find(a):
        while parent[a] != a:
            parent[a] = parent[parent[a]]
            a = parent[a]
        return a

    # edge list (local): row of each edge, col of each edge
    erow = np.repeat(np.arange(nrow), d)
    ecol = cols
    order = np.argsort(ecol, kind="stable")
    sc = ecol[order]
    sr = erow[order]
    # link consecutive same-col edges (covers all refs of each col)
    same = np.nonzero(sc[1:] == sc[:-1])[0]
    for i in same:
        a, b = find(sr[i]), find(sr[i + 1])
        if a == b:
            continue
        if crows[a] + crows[b] <= CROWS and csize[a] + csize[b] <= CSLOT:
            parent[b] = a
            csize[a] += csize[b]
            crows[a] += crows[b]
    groups = {}
    for r in range(nrow):
        if d[r] == 0:
            continue
        groups.setdefault(int(find(r)), []).append(r)
    return groups


def _prepare_core(rows, cols, vals, core):
    """Build items (clusters + splittable fragments) and bin-pack them."""
    lo = core * RPC
    bounds = np.searchsorted(rows, np.arange(lo, lo + RPC + 1))
    starts = bounds[:-1]
    d = (bounds[1:] - bounds[:-1]).astype(np.int64)
    cols32 = np.asarray(cols).astype(np.int32, copy=False)
    vals32 = np.asarray(vals).astype(np.float32, copy=False)

    groups = _cluster_rows(d, starts, cols32[bounds[0] : bounds[-1]], None)
    items = []
    for members in groups.values():
        if len(members) == 1:
            r = members[0]
            s0 = int(starts[r])
            deg = int(d[r])
            # split very long rows
            for off in range(0, deg, DMAX):
                take = min(DMAX, deg - off)
                items.append(
                    _Item(
                        cols32[s0 + off : s0 + off + take],
                        [(r, np.arange(take), vals32[s0 + off : s0 + off + take])],
                        True,
                    )
                )
        else:
            allc = np.concatenate(
                [cols32[starts[r] : starts[r] + d[r]] for r in members]
            )
            uniq, inv = np.unique(allc, return_inverse=True)
            rows_list = []
            off = 0
            for r in members:
                deg = int(d[r])
                rows_list.append(
                    (r, inv[off : off + deg], vals32[starts[r] : starts[r] + deg])
                )
                off += deg
            items.append(_Item(uniq, rows_list, False))
    return _pack_items(items), items


def _pack_items(items):
    """Greedy largest-fit packing of items into tiles (<=128 slots, <=M_FIX
    bval columns).  ALL items are splittable at slot granularity (a slot
    lands in exactly one piece; a row spanning pieces gets one bval column
    per piece and the host sums them), so tiles fill to exactly 128.
    Returns bins as lists of (item_id, slot_off, slot_take)."""
    maxd = max((len(it.slots) for it in items), default=0)
    by_size = [[] for _ in range(maxd + 1)]
    for i, it in enumerate(items):
        by_size[len(it.slots)].append(i)
    navail = len(items)
    used = {}
    bins = []
    while navail:
        cap = 128
        rows_left = M_FIX
        pieces = []
        cur = maxd
        while rows_left > 0 and navail and cap > 0:
            while cur > 0 and not by_size[cur]:
                cur -= 1
            if cur == 0:
                break
            dd = min(cap, cur)
            while dd > 0 and not by_size[dd]:
                dd -= 1
            # prefer the largest whole item whose rows also fit
            picked = None
            if dd > 0:
                cand = by_size[dd][-1]
                if len(items[cand].rows) <= rows_left:
                    picked = (dd, by_size[dd].pop())
            if picked is not None:
                sz, iid = picked
                navail -= 1
                off = used.get(iid, 0)
                used[iid] = off + sz
                pieces.append((iid, off, sz))
                cap -= sz
                rows_left -= len(items[iid].rows)
            else:
                # split the largest remaining item to fill the tile
                iid = by_size[cur].pop()
                it = items[iid]
                if len(it.rows) > rows_left:
                    # cannot even host its rows; close the tile
                    by_size[cur].append(iid)
                    break
                take = min(cap, cur)
                off = used.get(iid, 0)
                used[iid] = off + take
                pieces.append((iid, off, take))
                rem = cur - take
                if rem > 0:
                    by_size[rem].append(iid)
                else:
                    navail -= 1
                cap -= take
                rows_left -= len(it.rows)
        bins.append(pieces)
    return bins


def _assemble_core(bins, items, nt):
    idx_all = np.zeros((P, nt), np.int32)
    bval_all = np.zeros((P, nt * M_FIX), np.float32)
    prow, ppos = [], []
    for t, pieces in enumerate(bins):
        base = 0
        bcol = 0
        for iid, off, take in pieces:
            it = items[iid]
            idx_all[base : base + take, t] = it.slots[off : off + take]
            for r, sidx, rv in it.rows:
                sel = (sidx >= off) & (sidx < off + take)
                if not np.any(sel):
                    continue
                srel = sidx[sel] - off
                vsel = rv[sel]
                np.add.at(bval_all[:, t * M_FIX + bcol], base + srel, vsel)
                prow.append(int(r))
                ppos.append(t * M_FIX + bcol)
                bcol += 1
            base += take
    return idx_all, bval_all, np.asarray(prow, np.int64), np.asarray(ppos, np.int64)


def _build_program(nsb):
    nt = nsb * TPS
    nrows = nt * M_FIX
    nc = bacc.Bacc("TRN2", target_bir_lowering=False, debug=False)
    x = nc.dram_tensor("x", [N_NODES, F], f32, kind="ExternalInput")
    idx = nc.dram_tensor("idx", [P, nt], i32, kind="ExternalInput")
    bval = nc.dram_tensor("bval", [P, nrows], f32, kind="ExternalInput")
    w = nc.dram_tensor("w", [F, F], f32, kind="ExternalInput")
    out = nc.dram_tensor("out", [nrows, F], f32, kind="ExternalOutput")

    with tile.TileContext(nc) as tc:
        with (
            tc.tile_pool(name="const", bufs=1) as cpool,
            tc.tile_pool(name="meta", bufs=4) as mpool,
            tc.tile_pool(name="gath", bufs=16) as gpool,
            tc.tile_pool(name="zt", bufs=3, space="PSUM") as ztpool,
            tc.tile_pool(name="po", bufs=2, space="PSUM") as popool,
            tc.tile_pool(name="outp", bufs=3) as opool,
        ):
            wt = cpool.tile([F, F], f32)
            nc.sync.dma_start(wt[:], w[:])
            for sb in range(nsb):
                idx_t = mpool.tile([P, TPS], i32, tag="idx")
                nc.sync.dma_start(idx_t[:], idx[:, sb * TPS : (sb + 1) * TPS])
                bval_t = mpool.tile([P, RPS], f32, tag="bval")
                nc.sync.dma_start(bval_t[:], bval[:, sb * RPS : (sb + 1) * RPS])
                zt = ztpool.tile([F, RPS], f32, tag="zt")
                for t in range(TPS):
                    # HW-supported indirect mode: 128 per-partition offsets,
                    # one x-row (128B) per partition.
                    gath = gpool.tile([P, F], f32, tag="gath")
                    nc.gpsimd.indirect_dma_start(
                        out=gath[:],
                        out_offset=None,
                        in_=x[:],
                        in_offset=IndirectOffsetOnAxis(
                            ap=idx_t[:, t : t + 1], axis=0
                        ),
                    )
                    nc.tensor.matmul(
                        out=zt[:, t * M_FIX : (t + 1) * M_FIX],
                        lhsT=gath[:],
                        rhs=bval_t[:, t * M_FIX : (t + 1) * M_FIX],
                        start=True,
                        stop=True,
                    )
                zt_sb = opool.tile([F, RPS], f32, tag="ztsb")
                nc.vector.tensor_copy(zt_sb[:], zt[:])
                po = popool.tile([P, (RPS // P) * F], f32, tag="po")
                for c in range(RPS // P):
                    nc.tensor.matmul(
                        out=po[:, c * F : (c + 1) * F],
                        lhsT=zt_sb[:, c * P : (c + 1) * P],
                        rhs=wt[:],
                        start=True,
                        stop=True,
                    )
                ot = opool.tile([P, (RPS // P) * F], f32, tag="ot")
                nc.vector.tensor_copy(ot[:], po[:])
                nc.sync.dma_start(
                    out[sb * RPS : (sb + 1) * RPS, :].rearrange(
                        "(c p) f -> p c f", p=P
                    ),
                    ot[:].rearrange("p (c f) -> p c f", f=F),
                )
    nc.compile()
    return nc


def kernel(x, rows, cols, vals, weight):
    x = np.ascontiguousarray(np.asarray(x, dtype=np.float32))
    rows = np.asarray(rows)
    cols = np.asarray(cols)
    vals = np.asarray(vals, dtype=np.float32)
    weight = np.ascontiguousarray(np.asarray(weight, dtype=np.float32))

    per_core = [_prepare_core(rows, cols, vals, k) for k in range(N_CORES)]
    max_bins = max(len(pc[0]) for pc in per_core)
    nsb = max(1, (max_bins + TPS - 1) // TPS)
    nt = nsb * TPS

    if nsb not in _compiled_cache:
        _compiled_cache[nsb] = _build_program(nsb)
    nc = _compiled_cache[nsb]

    in_maps = []
    poss = []
    for k in range(N_CORES):
        bins, items = per_core[k]
        idx_all, bval_all, prow, ppos = _assemble_core(bins, items, nt)
        poss.append((prow, ppos))
        in_maps.append({"x": x, "idx": idx_all, "bval": bval_all, "w": weight})

    res = run_bass_kernel_spmd(nc, in_maps, list(range(N_CORES)))

    out_full = np.zeros((N_NODES, F), np.float32)
    for k in range(N_CORES):
        dev = res.results[k]["out"]
        prow, ppos = poss[k]
        # rows split into multiple pieces accumulate; others assign once
        np.add.at(out_full, k * RPC + prow, dev[ppos])
    return out_full



# revision 10
# speedup vs baseline: 1.1843x; 1.1843x over previous
"""GCNConv (COO SpMM + feature transform) distributed over 8 NeuronCores.

out = segment_sum(x[cols] * vals, rows) @ weight

Strategy (1D row partition per the CAGNET-style hint): core k owns dest rows
[12500k, 12500(k+1)) and the edges targeting them; x and the 32x32 weight are
replicated.

The kernel is bound by GpSimd SWDGE descriptor generation (~8.6ns per
gathered row, serial on the Pool engine), so v3 minimizes descriptor count
and keeps the Pool engine saturated:
 - Per (core, class=col%4): rows sharing class-cols are clustered (capped
   union-find) so edges with an identical col share ONE gather slot (the
   bval one-hot column for that slot simply has several nonzeros).  ~20%
   fewer descriptors than one-slot-per-edge.
 - Tiles: 128 slots, <=32 packed dest rows; tile t (class t%4) owns packed
   col range [32*(t%16), ...) of super-block t//16; 16 tiles accumulate into
   one PSUM bank zt[32, 512] (every column written, pad tiles write zeros).
 - All gather offsets live SBUF-resident (one load at startup), so the
   gathers issue back-to-back with no per-tile dependency stalls.
 - Eviction applies the weight (4 f32 matmuls) and stores a contiguous
   [128, 128] f32 block per super-block; the host un-permutes packed rows
   (np.add.at sums rows split across tiles/classes).
 - Program shape depends only on NSB -> one NEFF runs SPMD on all 8 cores.
"""

import os
import sys
import tempfile
import types

import numpy as np

# A transiently-wedged device can leave a poisoned NEFF in the shared neuron
# compile cache, making every later invocation with the same cache key crash
# (observed: NRT_EXEC_UNIT_UNRECOVERABLE on known-good programs).  Compiling
# is only a few seconds here, so use a fresh per-process cache instead.
os.environ["NEURON_COMPILE_CACHE_URL"] = tempfile.mkdtemp(prefix="neuron-cc-cache-")


def _install_ntff_hook_shim():
    """bass_utils' axon trace path imports antenv.axon_hooks, which this
    container image lacks.  Provide it (with the real ctypes-based profiler
    hook when available) so BASS_TRACE=1 in the environment doesn't crash."""
    if "antenv.axon_hooks" in sys.modules:
        return
    mod = types.ModuleType("antenv.axon_hooks")
    _h = [None]
    mod.set_axon_ntff_profile_hook = lambda h: _h.__setitem__(0, h)
    mod.get_axon_ntff_profile_hook = lambda: _h[0]
    sys.modules["antenv.axon_hooks"] = mod
    try:
        from trn_agent_boot.trn_boot import _ntff_profile_via_ctypes

        mod.set_axon_ntff_profile_hook(
            _ntff_profile_via_ctypes("/opt/axon/libaxon_pjrt.so")
        )
    except Exception:
        pass


_install_ntff_hook_shim()

import concourse.bass as bass
import concourse.mybir as mybir
import concourse.tile as tile
from concourse import bacc
from concourse.bass import IndirectOffsetOnAxis
from concourse.bass_utils import run_bass_kernel_spmd

N_NODES = 100_000
N_CORES = 8
RPC = N_NODES // N_CORES  # rows per core
F = 32
NCLS = 4                  # classes = col % 4
W = 64                    # packed output cols per tile (max rows per tile)
TPS = 8                   # tiles per super-block (8*64 = 512 PSUM cols)
P = 128                   # slots per tile

MAXR = 32                 # cluster cap: rows
MAXE = 128                # cluster cap: edges

f32 = mybir.dt.float32
i32 = mybir.dt.int32

_compiled_cache = {}


class _Item:
    """A packable cluster piece: `cols` (one gather slot per unique col) and
    CSR slot->edges arrays (row = GLOBAL dest row)."""

    __slots__ = ("cols", "eoff", "erow", "eval_", "nrows")

    def __init__(self, cols, eoff, erow, eval_, nrows):
        self.cols = cols
        self.eoff = eoff      # [len(cols)+1] edge offsets per slot
        self.erow = erow      # edge rows, grouped by slot
        self.eval_ = eval_
        self.nrows = nrows    # unique rows across all slots


def _split_item(it, cap, roomw):
    """Take the largest slot-prefix of `it` fitting (cap slots, roomw rows).
    Returns (piece, remainder|None)."""
    rows_seen = set()
    k = 0
    while k < len(it.cols) and k < cap:
        new = rows_seen | set(it.erow[it.eoff[k] : it.eoff[k + 1]].tolist())
        if len(new) > roomw:
            break
        rows_seen = new
        k += 1
    if k == 0:
        return None, it
    e1 = int(it.eoff[k])
    piece = _Item(
        it.cols[:k], it.eoff[: k + 1], it.erow[:e1], it.eval_[:e1], len(rows_seen)
    )
    if k == len(it.cols):
        return piece, None
    rerow = it.erow[e1:]
    rem = _Item(
        it.cols[k:], it.eoff[k:] - e1, rerow, it.eval_[e1:], len(np.unique(rerow))
    )
    return piece, rem


def _cluster_and_pack(rj, cj, vj):
    """Per (core, class): cluster rows sharing cols (capped union-find), then
    pack clusters into tiles of <=P slots / <=W rows with slot-granular
    splitting so tiles fill to exactly P.  A slot serves every clustered edge
    with that col (bval column gets several nonzeros).

    Returns list of tiles: (slots_cols, edge_slot, edge_row_global, edge_val)."""
    pres, inv_r, deg = np.unique(rj, return_inverse=True, return_counts=True)
    nr = len(pres)
    if nr == 0:
        return []
    parent = np.arange(nr)
    szr = np.ones(nr, np.int64)
    sze = deg.copy().astype(np.int64)

    def find(a):
        while parent[a] != a:
            parent[a] = parent[parent[a]]
            a = parent[a]
        return a

    order = np.argsort(cj, kind="stable")
    sc = cj[order]
    sr = inv_r[order]
    same = np.nonzero(sc[1:] == sc[:-1])[0]
    for i in same:
        a, b = find(sr[i]), find(sr[i + 1])
        if a == b:
            continue
        if szr[a] + szr[b] <= MAXR and sze[a] + sze[b] <= MAXE:
            parent[b] = a
            szr[a] += szr[b]
            sze[a] += sze[b]
    root = np.fromiter((find(i) for i in range(nr)), np.int64, nr)

    corder = np.argsort(root, kind="stable")
    croots = root[corder]
    starts = np.nonzero(np.concatenate([[True], croots[1:] != croots[:-1]]))[0]
    bounds = np.concatenate([starts, [nr]])

    er_order = np.argsort(inv_r, kind="stable")
    row_off = np.concatenate([[0], np.cumsum(deg)])
    items = []
    for gi in range(len(starts)):
        ranks = corder[bounds[gi] : bounds[gi + 1]]
        eidx = np.concatenate([er_order[row_off[r] : row_off[r + 1]] for r in ranks])
        ccols = cj[eidx]
        so = np.argsort(ccols, kind="stable")  # group edges by col
        scols = ccols[so]
        uniq_mask = np.concatenate([[True], scols[1:] != scols[:-1]])
        uniq = scols[uniq_mask]
        eoff = np.concatenate([np.nonzero(uniq_mask)[0], [len(scols)]])
        items.append(
            _Item(uniq, eoff, pres[inv_r[eidx[so]]], vj[eidx[so]], len(ranks))
        )

    # pack: by-size buckets, prefer largest whole item, split to fill
    maxd = max(len(it.cols) for it in items)
    by_size = [[] for _ in range(maxd + 1)]
    for it in items:
        by_size[len(it.cols)].append(it)
    navail = len(items)
    tiles = []
    while navail:
        cap = P
        roomw = W
        pieces = []
        cur = maxd
        while navail and cap > 0 and roomw > 0:
            while cur > 0 and not by_size[cur]:
                cur -= 1
            if cur == 0:
                break
            dd = min(cap, cur)
            while dd > 0 and not by_size[dd]:
                dd -= 1
            placed = False
            if dd > 0:
                cand = by_size[dd][-1]
                if cand.nrows <= roomw:
                    by_size[dd].pop()
                    navail -= 1
                    pieces.append(cand)
                    cap -= len(cand.cols)
                    roomw -= cand.nrows
                    placed = True
            if not placed:
                # split the largest remaining item to fill the tile
                it = by_size[cur].pop()
                piece, rem = _split_item(it, cap, roomw)
                if rem is not None:
                    by_size[len(rem.cols)].append(rem)
                    if len(rem.cols) > cur:
                        cur = len(rem.cols)
                else:
                    navail -= 1
                if piece is None:
                    break  # tile can't host even one slot of it
                pieces.append(piece)
                cap -= len(piece.cols)
                roomw -= piece.nrows
        if not pieces:
            break
        slots_cols = np.concatenate([p.cols for p in pieces])
        e_slot = np.concatenate(
            [
                np.repeat(np.arange(len(p.cols)) + so, np.diff(p.eoff))
                for p, so in zip(
                    pieces, np.cumsum([0] + [len(p.cols) for p in pieces])[:-1]
                )
            ]
        )
        e_row = np.concatenate([p.erow for p in pieces])
        e_val = np.concatenate([p.eval_ for p in pieces])
        tiles.append((slots_cols, e_slot, e_row, e_val))
    return tiles


def _prepare_core(rows, cols, vals, core):
    lo = core * RPC
    e0, e1 = np.searchsorted(rows, [lo, lo + RPC])
    r = (np.asarray(rows[e0:e1]) - lo).astype(np.int64)
    c = np.asarray(cols[e0:e1]).astype(np.int64)
    v = np.asarray(vals[e0:e1]).astype(np.float32)
    out = []
    for j in range(NCLS):
        m = c & 3 == j
        out.append(_cluster_and_pack(r[m], c[m].astype(np.int32), v[m]))
    return out


def _nsb_for(per_core):
    need = np.zeros(NCLS, np.int64)
    for pc in per_core:
        for j in range(NCLS):
            need[j] = max(need[j], len(pc[j]))
    nsb = 1
    while True:
        nt = nsb * TPS
        if all((nt + NCLS - 1 - j) // NCLS >= need[j] for j in range(NCLS)):
            return nsb
        nsb += 1


def _assemble_core(per_core_k, nsb):
    nt = nsb * TPS
    idx_all = np.zeros((128, nt), np.int32)
    bval = np.zeros((128, nsb * 512), np.float32)
    prow_l, ppos_l = [], []
    for j in range(NCLS):
        tiles = per_core_k[j]
        tids = np.arange(j, nt, NCLS)
        assert len(tiles) <= len(tids), (len(tiles), len(tids), j)
        for (scols, e_slot, e_row, e_val), t in zip(tiles, tids):
            s, tl = divmod(t, TPS)
            ns = len(scols)
            idx_all[:ns, t] = scols
            rws, e_rloc = np.unique(e_row, return_inverse=True)
            assert len(rws) <= W and ns <= P
            np.add.at(bval, (e_slot, s * 512 + tl * W + e_rloc), e_val)
            prow_l.append(rws)
            cc = tl * W + np.arange(len(rws))
            ppos_l.append((s * 128 + cc % 128) * 4 + cc // 128)
    prow = np.concatenate(prow_l) if prow_l else np.zeros(0, np.int64)
    ppos = np.concatenate(ppos_l) if ppos_l else np.zeros(0, np.int64)
    return idx_all, bval, prow, ppos


def _build_program(nsb):
    nt = nsb * TPS
    nc = bacc.Bacc("TRN2", target_bir_lowering=False, debug=False)
    x = nc.dram_tensor("x", [N_NODES, F], f32, kind="ExternalInput")
    idx = nc.dram_tensor("idx", [128, nt], i32, kind="ExternalInput")
    bval = nc.dram_tensor("bval", [128, nsb * 512], f32, kind="ExternalInput")
    w = nc.dram_tensor("w", [F, F], f32, kind="ExternalInput")
    out = nc.dram_tensor("out", [nsb, 128, 128], f32, kind="ExternalOutput")

    with tile.TileContext(nc) as tc:
        with (
            tc.tile_pool(name="const", bufs=1) as cpool,
            tc.tile_pool(name="bv", bufs=3) as bvpool,
            tc.tile_pool(name="g", bufs=3) as gpool,
            tc.tile_pool(name="zt", bufs=4, space="PSUM") as ztpool,
            tc.tile_pool(name="po", bufs=2, space="PSUM") as popool,
            tc.tile_pool(name="ev", bufs=3) as evpool,
        ):
            wt = cpool.tile([F, F], f32)
            nc.sync.dma_start(wt[:], w[:])
            # all gather offsets stay SBUF-resident: zero per-tile load stalls
            idxr = cpool.tile([128, nt], i32)
            nc.sync.dma_start(idxr[:], idx[:])
            for s in range(nsb):
                gt = gpool.tile([128, TPS * F], f32, tag="g")
                for tl in range(TPS):
                    t = s * TPS + tl
                    nc.gpsimd.indirect_dma_start(
                        out=gt[:, tl * F : (tl + 1) * F],
                        out_offset=None,
                        in_=x[:],
                        in_offset=IndirectOffsetOnAxis(
                            ap=idxr[:, t : t + 1], axis=0
                        ),
                    )
                bvt = bvpool.tile([128, 512], f32, tag="bv")
                nc.sync.dma_start(bvt[:], bval[:, s * 512 : (s + 1) * 512])
                zt = ztpool.tile([F, 512], f32, tag="zt")
                for tl in range(TPS):
                    nc.tensor.matmul(
                        out=zt[:, tl * W : (tl + 1) * W],
                        lhsT=gt[:, tl * F : (tl + 1) * F],
                        rhs=bvt[:, tl * W : (tl + 1) * W],
                        start=True,
                        stop=True,
                    )
                zsb = evpool.tile([F, 512], f32, tag="zsb")
                nc.vector.tensor_copy(zsb[:], zt[:])
                pot = popool.tile([128, 4 * F], f32, tag="po")
                for ci in range(4):
                    nc.tensor.matmul(
                        out=pot[:, ci * F : (ci + 1) * F],
                        lhsT=zsb[:, ci * 128 : (ci + 1) * 128],
                        rhs=wt[:],
                        start=True,
                        stop=True,
                    )
                ott = evpool.tile([128, 4 * F], f32, tag="ot")
                nc.vector.tensor_copy(ott[:], pot[:])
                nc.scalar.dma_start(out[s], ott[:])
    nc.compile()
    return nc


def prepare(x, rows, cols, vals, weight):
    """Host packing + compile.  Returns (nc, in_maps, maps, nsb)."""
    x = np.ascontiguousarray(np.asarray(x, dtype=np.float32))
    rows = np.asarray(rows)
    cols = np.asarray(cols)
    vals = np.asarray(vals, dtype=np.float32)
    weight = np.ascontiguousarray(np.asarray(weight, dtype=np.float32))

    per_core = [_prepare_core(rows, cols, vals, k) for k in range(N_CORES)]
    nsb = _nsb_for(per_core)

    if nsb not in _compiled_cache:
        _compiled_cache[nsb] = _build_program(nsb)
    nc = _compiled_cache[nsb]

    in_maps = []
    maps = []
    for k in range(N_CORES):
        idx_all, bval, prow, ppos = _assemble_core(per_core[k], nsb)
        maps.append((prow, ppos))
        in_maps.append({"x": x, "idx": idx_all, "bval": bval, "w": weight})
    return nc, in_maps, maps, nsb


def gather_output(res, maps, nsb):
    out_full = np.zeros((N_NODES, F), np.float32)
    for k in range(N_CORES):
        dev = np.asarray(res.results[k]["out"], dtype=np.float32)
        dev = dev.reshape(nsb * 128 * 4, 32)
        prow, ppos = maps[k]
        np.add.at(out_full, k * RPC + prow, dev[ppos])
    return out_full


def kernel(x, rows, cols, vals, weight):
    nc, in_maps, maps, nsb = prepare(x, rows, cols, vals, weight)
    res = run_bass_kernel_spmd(nc, in_maps, list(range(N_CORES)))
    return gather_output(res, maps, nsb)


# revision 11
# speedup vs baseline: 1.4333x; 1.2103x over previous
"""GCNConv (COO SpMM + feature transform) distributed over 8 NeuronCores.

out = segment_sum(x[cols] * vals, rows) @ weight

Strategy (1D row partition per the CAGNET-style hint): core k owns dest rows
[12500k, 12500(k+1)) and the edges targeting them; x and the 32x32 weight are
replicated.

The kernel is bound by GpSimd SWDGE descriptor generation (~8.6ns per
gathered row, serial on the Pool engine), so v3 minimizes descriptor count
and keeps the Pool engine saturated:
 - Per (core, class=col%4): rows sharing class-cols are clustered (capped
   union-find) so edges with an identical col share ONE gather slot (the
   bval one-hot column for that slot simply has several nonzeros).  ~20%
   fewer descriptors than one-slot-per-edge.
 - Tiles: 128 slots, <=32 packed dest rows; tile t (class t%4) owns packed
   col range [32*(t%16), ...) of super-block t//16; 16 tiles accumulate into
   one PSUM bank zt[32, 512] (every column written, pad tiles write zeros).
 - All gather offsets live SBUF-resident (one load at startup), so the
   gathers issue back-to-back with no per-tile dependency stalls.
 - Eviction applies the weight (4 f32 matmuls) and stores a contiguous
   [128, 128] f32 block per super-block; the host un-permutes packed rows
   (np.add.at sums rows split across tiles/classes).
 - Program shape depends only on NSB -> one NEFF runs SPMD on all 8 cores.
"""

import os
import sys
import tempfile
import types

import numpy as np

# A transiently-wedged device can leave a poisoned NEFF in the shared neuron
# compile cache, making every later invocation with the same cache key crash
# (observed: NRT_EXEC_UNIT_UNRECOVERABLE on known-good programs).  Compiling
# is only a few seconds here, so use a fresh per-process cache instead.
os.environ["NEURON_COMPILE_CACHE_URL"] = tempfile.mkdtemp(prefix="neuron-cc-cache-")


def _install_ntff_hook_shim():
    """bass_utils' axon trace path imports antenv.axon_hooks, which this
    container image lacks.  Provide it (with the real ctypes-based profiler
    hook when available) so BASS_TRACE=1 in the environment doesn't crash."""
    if "antenv.axon_hooks" in sys.modules:
        return
    mod = types.ModuleType("antenv.axon_hooks")
    _h = [None]
    mod.set_axon_ntff_profile_hook = lambda h: _h.__setitem__(0, h)
    mod.get_axon_ntff_profile_hook = lambda: _h[0]
    sys.modules["antenv.axon_hooks"] = mod
    try:
        from trn_agent_boot.trn_boot import _ntff_profile_via_ctypes

        mod.set_axon_ntff_profile_hook(
            _ntff_profile_via_ctypes("/opt/axon/libaxon_pjrt.so")
        )
    except Exception:
        pass


_install_ntff_hook_shim()

import concourse.bass as bass
import concourse.mybir as mybir
import concourse.tile as tile
from concourse import bacc
from concourse.bass import IndirectOffsetOnAxis
from concourse.bass_utils import run_bass_kernel_spmd

N_NODES = 100_000
N_CORES = 8
RPC = N_NODES // N_CORES  # rows per core
F = 32
NCLS = 8                  # classes = col % 8
W = 128                   # packed output cols per tile (max rows per tile)
TPS = 4                   # tiles per super-block (4*128 = 512 PSUM cols)
P = 128                   # slots per tile

MAXR = 110                # cluster cap: rows
MAXE = 128                # cluster cap: edges

f32 = mybir.dt.float32
i32 = mybir.dt.int32

_compiled_cache = {}


class _Item:
    """A packable cluster piece: `cols` (one gather slot per unique col) and
    CSR slot->edges arrays (row = GLOBAL dest row)."""

    __slots__ = ("cols", "eoff", "erow", "eval_", "nrows")

    def __init__(self, cols, eoff, erow, eval_, nrows):
        self.cols = cols
        self.eoff = eoff      # [len(cols)+1] edge offsets per slot
        self.erow = erow      # edge rows, grouped by slot
        self.eval_ = eval_
        self.nrows = nrows    # unique rows across all slots


def _split_item(it, cap, roomw):
    """Take the largest slot-prefix of `it` fitting (cap slots, roomw rows).
    Returns (piece, remainder|None)."""
    rows_seen = set()
    k = 0
    while k < len(it.cols) and k < cap:
        new = rows_seen | set(it.erow[it.eoff[k] : it.eoff[k + 1]].tolist())
        if len(new) > roomw:
            break
        rows_seen = new
        k += 1
    if k == 0:
        return None, it
    e1 = int(it.eoff[k])
    piece = _Item(
        it.cols[:k], it.eoff[: k + 1], it.erow[:e1], it.eval_[:e1], len(rows_seen)
    )
    if k == len(it.cols):
        return piece, None
    rerow = it.erow[e1:]
    rem = _Item(
        it.cols[k:], it.eoff[k:] - e1, rerow, it.eval_[e1:], len(np.unique(rerow))
    )
    return piece, rem


def _cluster_and_pack(rj, cj, vj):
    """Per (core, class): cluster rows sharing cols (capped union-find), then
    pack clusters into tiles of <=P slots / <=W rows with slot-granular
    splitting so tiles fill to exactly P.  A slot serves every clustered edge
    with that col (bval column gets several nonzeros).

    Returns list of tiles: (slots_cols, edge_slot, edge_row_global, edge_val)."""
    pres, inv_r, deg = np.unique(rj, return_inverse=True, return_counts=True)
    nr = len(pres)
    if nr == 0:
        return []
    parent = np.arange(nr)
    szr = np.ones(nr, np.int64)
    sze = deg.copy().astype(np.int64)

    def find(a):
        while parent[a] != a:
            parent[a] = parent[parent[a]]
            a = parent[a]
        return a

    order = np.argsort(cj, kind="stable")
    sc = cj[order]
    sr = inv_r[order]
    same = np.nonzero(sc[1:] == sc[:-1])[0]
    for i in same:
        a, b = find(sr[i]), find(sr[i + 1])
        if a == b:
            continue
        if szr[a] + szr[b] <= MAXR and sze[a] + sze[b] <= MAXE:
            parent[b] = a
            szr[a] += szr[b]
            sze[a] += sze[b]
    root = np.fromiter((find(i) for i in range(nr)), np.int64, nr)

    corder = np.argsort(root, kind="stable")
    croots = root[corder]
    starts = np.nonzero(np.concatenate([[True], croots[1:] != croots[:-1]]))[0]
    bounds = np.concatenate([starts, [nr]])

    er_order = np.argsort(inv_r, kind="stable")
    row_off = np.concatenate([[0], np.cumsum(deg)])
    items = []
    for gi in range(len(starts)):
        ranks = corder[bounds[gi] : bounds[gi + 1]]
        eidx = np.concatenate([er_order[row_off[r] : row_off[r + 1]] for r in ranks])
        ccols = cj[eidx]
        so = np.argsort(ccols, kind="stable")  # group edges by col
        scols = ccols[so]
        uniq_mask = np.concatenate([[True], scols[1:] != scols[:-1]])
        uniq = scols[uniq_mask]
        eoff = np.concatenate([np.nonzero(uniq_mask)[0], [len(scols)]])
        items.append(
            _Item(uniq, eoff, pres[inv_r[eidx[so]]], vj[eidx[so]], len(ranks))
        )

    # pack: by-size buckets, prefer largest whole item, split to fill
    maxd = max(len(it.cols) for it in items)
    by_size = [[] for _ in range(maxd + 1)]
    for it in items:
        by_size[len(it.cols)].append(it)
    navail = len(items)
    tiles = []
    while navail:
        cap = P
        roomw = W
        pieces = []
        cur = maxd
        while navail and cap > 0 and roomw > 0:
            while cur > 0 and not by_size[cur]:
                cur -= 1
            if cur == 0:
                break
            dd = min(cap, cur)
            while dd > 0 and not by_size[dd]:
                dd -= 1
            placed = False
            if dd > 0:
                cand = by_size[dd][-1]
                if cand.nrows <= roomw:
                    by_size[dd].pop()
                    navail -= 1
                    pieces.append(cand)
                    cap -= len(cand.cols)
                    roomw -= cand.nrows
                    placed = True
            if not placed:
                # split the largest remaining item to fill the tile
                it = by_size[cur].pop()
                piece, rem = _split_item(it, cap, roomw)
                if rem is not None:
                    by_size[len(rem.cols)].append(rem)
                    if len(rem.cols) > cur:
                        cur = len(rem.cols)
                else:
                    navail -= 1
                if piece is None:
                    break  # tile can't host even one slot of it
                pieces.append(piece)
                cap -= len(piece.cols)
                roomw -= piece.nrows
        if not pieces:
            break
        slots_cols = np.concatenate([p.cols for p in pieces])
        e_slot = np.concatenate(
            [
                np.repeat(np.arange(len(p.cols)) + so, np.diff(p.eoff))
                for p, so in zip(
                    pieces, np.cumsum([0] + [len(p.cols) for p in pieces])[:-1]
                )
            ]
        )
        e_row = np.concatenate([p.erow for p in pieces])
        e_val = np.concatenate([p.eval_ for p in pieces])
        tiles.append((slots_cols, e_slot, e_row, e_val))
    return tiles


def _prepare_core(rows, cols, vals, core):
    lo = core * RPC
    e0, e1 = np.searchsorted(rows, [lo, lo + RPC])
    r = (np.asarray(rows[e0:e1]) - lo).astype(np.int64)
    c = np.asarray(cols[e0:e1]).astype(np.int64)
    v = np.asarray(vals[e0:e1]).astype(np.float32)
    out = []
    for j in range(NCLS):
        m = c % NCLS == j
        out.append(_cluster_and_pack(r[m], c[m].astype(np.int32), v[m]))
    return out


def _nsb_for(per_core):
    need = np.zeros(NCLS, np.int64)
    for pc in per_core:
        for j in range(NCLS):
            need[j] = max(need[j], len(pc[j]))
    nsb = 1
    while True:
        nt = nsb * TPS
        if all((nt + NCLS - 1 - j) // NCLS >= need[j] for j in range(NCLS)):
            return nsb
        nsb += 1


def _assemble_core(per_core_k, nsb):
    nt = nsb * TPS
    idx_all = np.zeros((128, nt), np.int32)
    bval = np.zeros((128, nsb * 512), np.float32)
    prow_l, ppos_l = [], []
    for j in range(NCLS):
        tiles = per_core_k[j]
        tids = np.arange(j, nt, NCLS)
        assert len(tiles) <= len(tids), (len(tiles), len(tids), j)
        for (scols, e_slot, e_row, e_val), t in zip(tiles, tids):
            s, tl = divmod(t, TPS)
            ns = len(scols)
            idx_all[:ns, t] = scols
            rws, e_rloc = np.unique(e_row, return_inverse=True)
            assert len(rws) <= W and ns <= P
            np.add.at(bval, (e_slot, s * 512 + tl * W + e_rloc), e_val)
            prow_l.append(rws)
            cc = tl * W + np.arange(len(rws))
            ppos_l.append((s * 128 + cc % 128) * 4 + cc // 128)
    prow = np.concatenate(prow_l) if prow_l else np.zeros(0, np.int64)
    ppos = np.concatenate(ppos_l) if ppos_l else np.zeros(0, np.int64)
    return idx_all, bval, prow, ppos


def _build_program(nsb):
    nt = nsb * TPS
    nc = bacc.Bacc("TRN2", target_bir_lowering=False, debug=False)
    x = nc.dram_tensor("x", [N_NODES, F], f32, kind="ExternalInput")
    idx = nc.dram_tensor("idx", [128, nt], i32, kind="ExternalInput")
    bval = nc.dram_tensor("bval", [128, nsb * 512], f32, kind="ExternalInput")
    w = nc.dram_tensor("w", [F, F], f32, kind="ExternalInput")
    out = nc.dram_tensor("out", [nsb, 128, 128], f32, kind="ExternalOutput")

    with tile.TileContext(nc) as tc:
        with (
            tc.tile_pool(name="const", bufs=1) as cpool,
            tc.tile_pool(name="bv", bufs=4) as bvpool,
            tc.tile_pool(name="g", bufs=6) as gpool,
            tc.tile_pool(name="zt", bufs=4, space="PSUM") as ztpool,
            tc.tile_pool(name="po", bufs=2, space="PSUM") as popool,
            tc.tile_pool(name="ev", bufs=4) as evpool,
        ):
            wt = cpool.tile([F, F], f32)
            nc.sync.dma_start(wt[:], w[:])
            # all gather offsets stay SBUF-resident: zero per-tile load stalls
            idxr = cpool.tile([128, nt], i32)
            nc.sync.dma_start(idxr[:], idx[:])
            for s in range(nsb):
                gt = gpool.tile([128, TPS * F], f32, tag="g")
                for tl in range(TPS):
                    t = s * TPS + tl
                    nc.gpsimd.indirect_dma_start(
                        out=gt[:, tl * F : (tl + 1) * F],
                        out_offset=None,
                        in_=x[:],
                        in_offset=IndirectOffsetOnAxis(
                            ap=idxr[:, t : t + 1], axis=0
                        ),
                    )
                bvt = bvpool.tile([128, 512], f32, tag="bv")
                nc.sync.dma_start(bvt[:], bval[:, s * 512 : (s + 1) * 512])
                zt = ztpool.tile([F, 512], f32, tag="zt")
                for tl in range(TPS):
                    nc.tensor.matmul(
                        out=zt[:, tl * W : (tl + 1) * W],
                        lhsT=gt[:, tl * F : (tl + 1) * F],
                        rhs=bvt[:, tl * W : (tl + 1) * W],
                        start=True,
                        stop=True,
                    )
                zsb = evpool.tile([F, 512], f32, tag="zsb")
                nc.vector.tensor_copy(zsb[:], zt[:])
                pot = popool.tile([128, 4 * F], f32, tag="po")
                for ci in range(4):
                    nc.tensor.matmul(
                        out=pot[:, ci * F : (ci + 1) * F],
                        lhsT=zsb[:, ci * 128 : (ci + 1) * 128],
                        rhs=wt[:],
                        start=True,
                        stop=True,
                    )
                ott = evpool.tile([128, 4 * F], f32, tag="ot")
                nc.vector.tensor_copy(ott[:], pot[:])
                nc.scalar.dma_start(out[s], ott[:])
    nc.compile()
    return nc


def prepare(x, rows, cols, vals, weight):
    """Host packing + compile.  Returns (nc, in_maps, maps, nsb)."""
    x = np.ascontiguousarray(np.asarray(x, dtype=np.float32))
    rows = np.asarray(rows)
    cols = np.asarray(cols)
    vals = np.asarray(vals, dtype=np.float32)
    weight = np.ascontiguousarray(np.asarray(weight, dtype=np.float32))

    per_core = [_prepare_core(rows, cols, vals, k) for k in range(N_CORES)]
    nsb = _nsb_for(per_core)

    if nsb not in _compiled_cache:
        _compiled_cache[nsb] = _build_program(nsb)
    nc = _compiled_cache[nsb]

    in_maps = []
    maps = []
    for k in range(N_CORES):
        idx_all, bval, prow, ppos = _assemble_core(per_core[k], nsb)
        maps.append((prow, ppos))
        in_maps.append({"x": x, "idx": idx_all, "bval": bval, "w": weight})
    return nc, in_maps, maps, nsb


def gather_output(res, maps, nsb):
    out_full = np.zeros((N_NODES, F), np.float32)
    for k in range(N_CORES):
        dev = np.asarray(res.results[k]["out"], dtype=np.float32)
        dev = dev.reshape(nsb * 128 * 4, 32)
        prow, ppos = maps[k]
        np.add.at(out_full, k * RPC + prow, dev[ppos])
    return out_full


def kernel(x, rows, cols, vals, weight):
    nc, in_maps, maps, nsb = prepare(x, rows, cols, vals, weight)
    res = run_bass_kernel_spmd(nc, in_maps, list(range(N_CORES)))
    return gather_output(res, maps, nsb)
